# revision 1
# baseline (speedup 1.0000x reference)
"""PlainGCN message passing on 8 TRN2 NeuronCores.

Computation (reference):
    deg = bincount(h); dis = deg**-0.5; norm = dis[t]*dis[h]
    out = relu(segment_sum(norm[:,None] * x[h], t, N))

Strategy:
  - Shard edges by destination node: core c owns dest nodes
    [c*N/8, (c+1)*N/8) and all edges targeting them. x replicated.
  - norm / degree computed host-side (edge metadata, like the sharding
    hint's "shard edges (h, t, norm, ...)").
  - On device, per core: dma_gather x rows by h (int16 indices =>
    4 source buckets of 32768 rows), build one-hot(dest-in-tile)*norm
    matrices on DVE, segment-reduce via TensorE matmul accumulation in
    PSUM per 128-dest-row tile, ReLU on ScalarE, DMA out.
  - SPMD: all 8 cores share one program. Per-(tile,bucket) run lengths
    are padded to the max across cores so the static schedule is shared;
    pad edges have norm=0 (gather idx 0) and contribute nothing.
"""

import numpy as np

import concourse.bacc as bacc
import concourse.bass as bass
import concourse.mybir as mybir
import concourse.tile as tile
from concourse.bass_utils import run_bass_kernel_spmd
from concourse.library_config import mlp as mlp_lib

P = 128


def _preprocess(x, h, t, n_cores, bucket_bits, tiles_per_group):
    """Host-side edge sharding + schedule construction.

    Returns (schedule, per_core_inputs, meta) where schedule is shared by
    all cores (shapes/segment structure identical) and per_core_inputs
    holds each core's data arrays.
    """
    n, d = x.shape
    e = h.shape[0]
    assert n % n_cores == 0
    npc = n // n_cores  # nodes per core
    n_tiles = -(-npc // P)  # dest tiles per core
    bucket = 1 << bucket_bits
    n_buckets = -(-n // bucket)

    h = h.astype(np.int64)
    t = t.astype(np.int64)

    deg = np.bincount(h, minlength=n).astype(np.float32)
    # deg >= 1 guaranteed by problem setup; guard anyway (0-deg source
    # nodes never appear in h so their dis value is never used).
    dis = np.where(deg > 0, deg, 1).astype(np.float32) ** np.float32(-0.5)
    norm = (dis[t] * dis[h]).astype(np.float32)

    core = t // npc
    tloc = t - core * npc
    j = tloc // P  # dest tile within core
    tin = (tloc % P).astype(np.float32)
    b = (h >> bucket_bits).astype(np.int64)
    gidx_all = (h - (b << bucket_bits)).astype(np.int16)

    # run_len[j, b] = max over cores of per-(core,j,b) edge count, padded
    # to a multiple of 64 so every matmul segment starts at partition 0
    # or 64 (PE base-partition constraint: must be 0, 32, or 64).
    counts = np.zeros((n_cores, n_tiles, n_buckets), dtype=np.int64)
    np.add.at(counts, (core, j, b), 1)
    run_len = counts.max(axis=0)  # [n_tiles, n_buckets]
    run_len = -(-run_len // 64) * 64

    n_groups = -(-n_tiles // tiles_per_group)
    groups = [
        list(range(g * tiles_per_group, min((g + 1) * tiles_per_group, n_tiles)))
        for g in range(n_groups)
    ]

    # span lengths (shared): span (g, b) covers runs (j in groups[g], b),
    # padded to a multiple of P.
    spans = []  # (g, b, start, length) in stream coords
    seg_lists = [[] for _ in range(n_tiles)]  # per tile: (col, p0, k, b)
    pos = 0
    for g, tiles_g in enumerate(groups):
        for bb in range(n_buckets):
            s0 = pos
            for jj in tiles_g:
                r = int(run_len[jj, bb])
                # split run [pos, pos+r) at column boundaries; all pieces
                # start at partition 0 or 64 with k in {64, 128}
                q = pos
                while q < pos + r:
                    k = min(P - (q % P), pos + r - q)
                    assert q % P in (0, 64) and k in (64, P)
                    seg_lists[jj].append((q // P, q % P, k, bb))
                    q += k
                pos += r
            pos = -(-pos // P) * P  # pad span to multiple of P
            spans.append((g, bb, s0, pos - s0))
    e_pad = pos
    n_cols = e_pad // P

    # Per-core data arrays in stream order
    per_core = []
    order_key = (((core * n_groups * n_buckets) + (j // tiles_per_group) * n_buckets + b)
                 * n_tiles + j)
    sort_idx = np.argsort(order_key, kind="stable")
    cum = np.zeros((n_cores, n_tiles, n_buckets), dtype=np.int64)
    for c in range(n_cores):
        gi = np.zeros(e_pad, dtype=np.int16)
        tf = np.zeros(e_pad, dtype=np.float32)
        nf = np.zeros(e_pad, dtype=np.float32)
        sel = sort_idx[core[sort_idx] == c]
        # place this core's edges run by run into the padded stream
        # compute per-edge destination offset in stream
        jj = j[sel]
        bb2 = b[sel]
        # run start offsets in the padded stream
        run_start = np.zeros((n_tiles, n_buckets), dtype=np.int64)
        for g, tiles_g in enumerate(groups):
            for bx in range(n_buckets):
                s0 = next(s0_ for (gg, bq, s0_, _l) in spans
                          if gg == g and bq == bx)
                acc = s0
                for jx in tiles_g:
                    run_start[jx, bx] = acc
                    acc += int(run_len[jx, bx])
        # offsets within run: stable order of appearance
        within = np.zeros(len(sel), dtype=np.int64)
        cnt = {}
        key = jj * n_buckets + bb2
        # vectorized "rank within group" for sorted keys (sel is sorted by key)
        change = np.r_[True, key[1:] != key[:-1]]
        grp_id = np.cumsum(change) - 1
        first_pos = np.r_[np.nonzero(change)[0]]
        within = np.arange(len(sel)) - first_pos[grp_id]
        posn = run_start[jj, bb2] + within
        gi[posn] = gidx_all[sel]
        tf[posn] = tin[sel]
        nf[posn] = norm[sel]

        # wrap gather indices: per span, index l -> [l%16, l//16], tiled x8
        wrap = np.zeros((P, e_pad // 16), dtype=np.int16)
        for (_g, _b, s0, ln) in spans:
            w0 = s0 // 16
            seg = gi[s0:s0 + ln].reshape(ln // 16, 16).T  # [16, ln/16]
            wrap[:, w0:w0 + ln // 16] = np.tile(seg, (8, 1))

        tlocF = tf.reshape(n_cols, P).T.copy()  # [128, C]
        normF = nf.reshape(n_cols, P).T.copy()
        meta = np.concatenate([tlocF, normF], axis=1)  # [128, 2C]
        per_core.append({"gidx": wrap, "meta": meta})

    iota = np.tile(np.arange(P, dtype=np.float32), (P, 1))  # [128,128] iota[p,f]=f

    schedule = {
        "n": n, "d": d, "npc": npc, "n_tiles": n_tiles, "n_cols": n_cols,
        "e_pad": e_pad, "bucket": bucket, "n_buckets": n_buckets,
        "groups": groups, "spans": spans, "seg_lists": seg_lists,
        "run_len": run_len,
    }
    return schedule, per_core, iota


def _build_program(sched, n_cores, stage="full"):
    n, d, npc = sched["n"], sched["d"], sched["npc"]
    n_tiles, n_cols, e_pad = sched["n_tiles"], sched["n_cols"], sched["e_pad"]
    bucket, n_buckets = sched["bucket"], sched["n_buckets"]
    groups, spans, seg_lists = sched["groups"], sched["spans"], sched["seg_lists"]

    nc = bacc.Bacc("TRN2", target_bir_lowering=False, debug=False,
                   num_devices=n_cores)
    f32 = mybir.dt.float32
    x_d = nc.dram_tensor("x", [n, d], f32, kind="ExternalInput")
    iota_d = nc.dram_tensor("iota", [P, P], f32, kind="ExternalInput")
    gidx_d = nc.dram_tensor("gidx", [P, e_pad // 16], mybir.dt.int16,
                            kind="ExternalInput")
    meta_d = nc.dram_tensor("meta", [P, 2 * n_cols], f32, kind="ExternalInput")
    y_d = nc.dram_tensor("y", [npc, d], f32, kind="ExternalOutput")

    nc.gpsimd.load_library(mlp_lib)

    max_span = max(ln for (_g, _b, _s, ln) in spans)
    span_by_gb = {(g, b): (s0, ln) for (g, b, s0, ln) in spans}

    with tile.TileContext(nc) as tc:
        with (
            tc.tile_pool(name="const", bufs=1) as cpool,
            tc.tile_pool(name="gather", bufs=6) as gpool,
            tc.tile_pool(name="onehot", bufs=16) as opool,
            tc.tile_pool(name="psum", bufs=8, space="PSUM") as ppool,
            tc.tile_pool(name="outs", bufs=4) as ypool,
        ):
            iota_t = cpool.tile([P, P], f32, tag="iota")
            nc.sync.dma_start(iota_t[:], iota_d[:, :])
            meta_t = cpool.tile([P, 2 * n_cols], f32, tag="meta")
            nc.sync.dma_start(meta_t[:], meta_d[:, :])
            gidx_t = cpool.tile([P, e_pad // 16], mybir.dt.int16, tag="gidx")
            nc.sync.dma_start(gidx_t[:], gidx_d[:, :])

            for g, tiles_g in enumerate(groups):
                # gathers for this group's spans
                gtiles = {}
                for b in range(n_buckets):
                    s0, ln = span_by_gb[(g, b)]
                    if ln == 0:
                        continue
                    base = b * bucket
                    rows = min(bucket, n - base)
                    gt = gpool.tile([P, (max_span // P) * d], f32, tag="gt", name=f"gt{g}_{b}")
                    gt_3d = gt[:, :(ln // P) * d].rearrange(
                        "p (c d) -> p c d", d=d
                    )
                    nc.gpsimd.dma_gather(
                        gt_3d,
                        x_d[base:base + rows, :],
                        gidx_t[:, s0 // 16:(s0 + ln) // 16],
                        ln, ln, d,
                        single_packet=(ln <= 1024),
                    )
                    gtiles[b] = (gt, s0)

                if stage == "gather":
                    # consume gather tiles minimally: copy first column out
                    for jj in tiles_g:
                        rows = min(P, npc - jj * P)
                        yt = ypool.tile([P, d], f32, tag="yt", name=f"yt{jj}")
                        gt0, _ = gtiles[0]
                        nc.vector.tensor_copy(yt[:], gt0[:, :d])
                        nc.sync.dma_start(y_d[jj * P:jj * P + rows, :],
                                          yt[:rows, :])
                    continue

                # onehot build + matmuls; PSUM sub-groups of 4 dest tiles
                # (each tile may need 2 PSUM banks: base-0 and base-64
                # accumulation chains — PE crashes if the operand base
                # partition changes inside one accumulation group).
                oh_tiles = {}

                def build_oh(col):
                    if col not in oh_tiles:
                        oh = opool.tile([P, P], f32, tag="oh",
                                        name=f"oh{col}")
                        nc.vector.tensor_scalar(
                            oh[:], iota_t[:],
                            meta_t[:, col:col + 1],
                            meta_t[:, n_cols + col:n_cols + col + 1],
                            mybir.AluOpType.is_equal,
                            mybir.AluOpType.mult,
                        )
                        oh_tiles[col] = oh
                    return oh_tiles[col]

                if stage == "onehot":
                    for jj in tiles_g:
                        for (col, p0, k, b) in seg_lists[jj]:
                            build_oh(col)
                        rows = min(P, npc - jj * P)
                        yt = ypool.tile([P, d], f32, tag="yt",
                                        name=f"yt{jj}")
                        oh_any = next(iter(oh_tiles.values()))
                        nc.vector.tensor_copy(yt[:], oh_any[:])
                        nc.sync.dma_start(y_d[jj * P:jj * P + rows, :],
                                          yt[:rows, :])
                    continue

                for sub0 in range(0, len(tiles_g), 4):
                    for jj in tiles_g[sub0:sub0 + 4]:
                        segs = seg_lists[jj]
                        ps = {}
                        for base in (0, 64):
                            ss = [s for s in segs if s[1] == base]
                            if not ss:
                                continue
                            pt = ppool.tile([P, d], f32, tag="ps",
                                            name=f"ps{jj}_{base}")
                            ps[base] = pt
                            for si, (col, p0, k, b) in enumerate(ss):
                                oh = build_oh(col)
                                gt, s0 = gtiles[b]
                                col_l = col - s0 // P
                                nc.tensor.matmul(
                                    pt[:],
                                    lhsT=oh[p0:p0 + k, :],
                                    rhs=gt[p0:p0 + k,
                                           col_l * d:(col_l + 1) * d],
                                    start=(si == 0),
                                    stop=(si == len(ss) - 1),
                                )
                        rows = min(P, npc - jj * P)
                        yt = ypool.tile([P, d], f32, tag="yt",
                                        name=f"yt{jj}")
                        relu = mybir.ActivationFunctionType.Relu
                        if 0 in ps and 64 in ps:
                            s64 = ypool.tile([P, d], f32, tag="s64",
                                             name=f"s64_{jj}")
                            nc.scalar.activation(
                                s64[:], ps[64][:],
                                mybir.ActivationFunctionType.Identity)
                            st = ypool.tile([P, d], f32, tag="st",
                                            name=f"st{jj}")
                            nc.vector.tensor_add(st[:], s64[:], ps[0][:])
                            nc.scalar.activation(yt[:], st[:], relu)
                        elif 0 in ps:
                            nc.scalar.activation(yt[:], ps[0][:], relu)
                        elif 64 in ps:
                            nc.scalar.activation(yt[:], ps[64][:], relu)
                        else:
                            nc.vector.memset(yt[:], 0.0)
                        nc.sync.dma_start(y_d[jj * P:jj * P + rows, :],
                                          yt[:rows, :])

    nc.compile()
    return nc


def _run(x, h, t, n_cores=8, bucket_bits=15, tiles_per_group=8, trace=False):
    import time
    t0 = time.monotonic()
    sched, per_core, iota = _preprocess(x, h, t, n_cores, bucket_bits,
                                        tiles_per_group)
    t1 = time.monotonic()
    print(f"[kernel] preprocess {t1 - t0:.1f}s  e_pad={sched['e_pad']} "
          f"cols={sched['n_cols']}", flush=True)
    nc = _build_program(sched, n_cores)
    t2 = time.monotonic()
    print(f"[kernel] build+tile-schedule {t2 - t1:.1f}s", flush=True)
    in_maps = [
        {"x": np.ascontiguousarray(x), "iota": iota,
         "gidx": pc["gidx"], "meta": pc["meta"]}
        for pc in per_core
    ]
    res = run_bass_kernel_spmd(nc, in_maps, core_ids=list(range(n_cores)),
                               trace=trace)
    t3 = time.monotonic()
    print(f"[kernel] compile+run {t3 - t2:.1f}s", flush=True)
    y = np.concatenate([res.results[c]["y"] for c in range(n_cores)], axis=0)
    return y, res


def kernel(x, h, t):
    y, _ = _run(np.asarray(x), np.asarray(h), np.asarray(t))
    return y



# revision 9
# speedup vs baseline: 1.4446x; 1.4446x over previous
"""PlainGCN message passing on 8 TRN2 NeuronCores.

Computation (reference):
    deg = bincount(h); dis = deg**-0.5; norm = dis[t]*dis[h]
    out = relu(segment_sum(norm[:,None] * x[h], t, N))

Strategy (v2):
  - Fold dis[h] into x host-side: x2 = dis[:,None]*x (bf16). Then
    out[t] = relu(dis[t] * segment_sum(x2[h], t)) — the per-edge norm
    disappears; dis[t] is applied once per dest tile, fused with the
    ReLU on ScalarE (per-partition scale).
  - Shard edges by destination: core c owns dest nodes
    [c*N/8, (c+1)*N/8); x2 replicated in HBM.
  - Per (dest tile j, source bucket b) the edges form a run padded to
    a multiple of 128 slots (shared SPMD schedule = max over cores).
    Pad slots carry gather idx -1 (skipped by the DMA when trailing)
    and tloc -1 (one-hot row = 0).
  - dma_gather x2 rows (256 B bf16) in <=1024-idx chunks,
    single_packet=True, round-robin over 4 SWDGE queues. The gather is
    Q7 descriptor-generation bound (~4 ns/idx), so everything else is
    scheduled to hide under it.
  - Segment-sum per dest tile: one-hot(tloc)=is_equal(iota) on DVE
    (bf16), TensorE matmul accumulation of full 128-slot columns into
    one PSUM bank per tile, then Relu(dis_t * psum) on ScalarE, DMA out.
"""

import numpy as np

import concourse.bacc as bacc
import concourse.mybir as mybir
import concourse.tile as tile
from concourse.bass_utils import run_bass_kernel_spmd
from concourse.library_config import mlp as mlp_lib

P = 128
N_NODES = 100000
D_FEAT = 128
N_CORES = 8
BUCKET_BITS = 15
TILE_BLOCK = 4       # dest tiles per gather block
GATHER_CHUNK = 1024  # max idxs per dma_gather (single_packet safe limit)


def _preprocess(x, h, t):
    n, d = x.shape
    assert (n, d) == (N_NODES, D_FEAT)
    npc = n // N_CORES
    n_tiles = -(-npc // P)
    bucket = 1 << BUCKET_BITS
    n_buckets = -(-n // bucket)

    h = h.astype(np.int64)
    t = t.astype(np.int64)

    deg = np.bincount(h, minlength=n).astype(np.float64)
    dis = np.where(deg > 0, deg, 1.0) ** -0.5
    x2 = (x.astype(np.float64) * dis[:, None]).astype(np.float32)

    core = t // npc
    tloc = t - core * npc
    j = tloc // P
    tin = (tloc % P).astype(np.float32)
    b = h >> BUCKET_BITS
    gidx_all = (h - (b << BUCKET_BITS)).astype(np.int16)

    counts = np.zeros((N_CORES, n_tiles, n_buckets), dtype=np.int64)
    np.add.at(counts, (core, j, b), 1)
    run_len = counts.max(axis=0)
    run_len = -(-run_len // P) * P  # full 128-slot columns only

    # stream order: blocks of TILE_BLOCK dest tiles; per block, per bucket,
    # the tiles' runs back to back.
    n_blocks = -(-n_tiles // TILE_BLOCK)
    run_start = np.zeros((n_tiles, n_buckets), dtype=np.int64)
    gathers = []  # (bucket, start, length) — <=GATHER_CHUNK, 128-aligned
    pos = 0
    for blk in range(n_blocks):
        tiles_blk = range(blk * TILE_BLOCK, min((blk + 1) * TILE_BLOCK, n_tiles))
        for bb in range(n_buckets):
            s0 = pos
            for jj in tiles_blk:
                run_start[jj, bb] = pos
                pos += int(run_len[jj, bb])
            # split [s0, pos) into gather chunks at run boundaries
            c0 = s0
            for jj in tiles_blk:
                r = int(run_len[jj, bb])
                end = run_start[jj, bb] + r
                if end - c0 > GATHER_CHUNK:
                    if run_start[jj, bb] > c0:
                        gathers.append((bb, c0, int(run_start[jj, bb] - c0)))
                    c0 = int(run_start[jj, bb])
                    while end - c0 > GATHER_CHUNK:
                        gathers.append((bb, c0, GATHER_CHUNK))
                        c0 += GATHER_CHUNK
            if pos > c0:
                gathers.append((bb, c0, int(pos - c0)))
    e_pad = pos
    n_cols = e_pad // P
    assert all(ln <= GATHER_CHUNK and ln % P == 0 for (_b, _s, ln) in gathers)

    # per-tile column lists: (col, bucket)
    tile_cols = []
    for jj in range(n_tiles):
        cols = []
        for bb in range(n_buckets):
            s, r = int(run_start[jj, bb]), int(run_len[jj, bb])
            cols.extend((c, bb) for c in range(s // P, (s + r) // P))
        tile_cols.append(cols)

    # per-core streams
    order_key = (j // TILE_BLOCK) * (n_buckets * n_tiles) + b * n_tiles + j
    per_core = []
    for c in range(N_CORES):
        sel = np.nonzero(core == c)[0]
        sel = sel[np.argsort(order_key[sel], kind="stable")]
        jj = j[sel]
        bb2 = b[sel]
        key = jj * n_buckets + bb2
        change = np.r_[True, key[1:] != key[:-1]]
        grp_id = np.cumsum(change) - 1
        first_pos = np.nonzero(change)[0]
        within = np.arange(len(sel)) - first_pos[grp_id]
        posn = run_start[jj, bb2] + within

        gi = np.full(e_pad, -1, dtype=np.int16)
        tf = np.full(e_pad, -1.0, dtype=np.float32)
        gi[posn] = gidx_all[sel]
        tf[posn] = tin[sel]
        # pad slots gather row 0 (harmless: their one-hot row is zero via
        # tloc=-1). Runtime -1-stripping desyncs the SWDGE ring bookkeeping
        # (device abort) — do not use negative indices.
        gi[gi == -1] = 0

        # wrap gather indices: [16, e/16] tiled x8 -> [128, e/16]
        wrap = np.tile(gi.reshape(e_pad // 16, 16).T, (8, 1)).astype(np.int16)

        meta = tf.reshape(n_cols, P).T.copy()  # [128, C] fp32

        # dis of this core's dest nodes, tiled [128, n_tiles]
        dnode = np.zeros(n_tiles * P, dtype=np.float32)
        dnode[:npc] = dis[c * npc:(c + 1) * npc].astype(np.float32)
        dis_t = dnode.reshape(n_tiles, P).T.copy()  # [128, n_tiles]

        per_core.append({"gidx": wrap, "meta": meta, "dis": dis_t})

    import ml_dtypes
    iota = np.tile(np.arange(P, dtype=np.float32), (P, 1)).astype(
        ml_dtypes.bfloat16)  # [128,128] iota[p,f]=f
    x2b = x2.astype(ml_dtypes.bfloat16)

    sched = {
        "n": n, "d": d, "npc": npc, "n_tiles": n_tiles, "n_cols": n_cols,
        "e_pad": e_pad, "bucket": bucket, "n_buckets": n_buckets,
        "n_blocks": n_blocks, "gathers": gathers, "tile_cols": tile_cols,
        "run_start": run_start, "run_len": run_len,
    }
    return sched, per_core, x2b, iota


def _build_program(sched, stage="full"):
    n, d, npc = sched["n"], sched["d"], sched["npc"]
    n_tiles, n_cols, e_pad = sched["n_tiles"], sched["n_cols"], sched["e_pad"]
    bucket, n_buckets = sched["bucket"], sched["n_buckets"]
    n_blocks, gathers = sched["n_blocks"], sched["gathers"]
    tile_cols = sched["tile_cols"]

    nc = bacc.Bacc("TRN2", target_bir_lowering=False, debug=False,
                   num_devices=N_CORES, num_swdge_queues=4)
    f32 = mybir.dt.float32
    bf16 = mybir.dt.bfloat16
    x_d = nc.dram_tensor("x2", [n, d], bf16, kind="ExternalInput")
    iota_d = nc.dram_tensor("iota", [P, P], bf16, kind="ExternalInput")
    gidx_d = nc.dram_tensor("gidx", [P, e_pad // 16], mybir.dt.int16,
                            kind="ExternalInput")
    meta_d = nc.dram_tensor("meta", [P, n_cols], f32, kind="ExternalInput")
    dis_d = nc.dram_tensor("dis", [P, n_tiles], f32, kind="ExternalInput")
    y_d = nc.dram_tensor("y", [npc, d], f32, kind="ExternalOutput")

    nc.gpsimd.load_library(mlp_lib)

    # gathers grouped by block for scheduling
    gather_of_col = {}
    for gid, (bb, s0, ln) in enumerate(gathers):
        for cc in range(s0 // P, (s0 + ln) // P):
            gather_of_col[cc] = (gid, s0 // P)

    relu = mybir.ActivationFunctionType.Relu

    with tile.TileContext(nc) as tc:
        with (
            tc.tile_pool(name="const", bufs=1) as cpool,
            tc.tile_pool(name="gather", bufs=10) as gpool,
            tc.tile_pool(name="onehot", bufs=12) as opool,
            tc.tile_pool(name="psum", bufs=8, space="PSUM") as ppool,
            tc.tile_pool(name="outs", bufs=4) as ypool,
        ):
            iota_t = cpool.tile([P, P], bf16, tag="iota")
            nc.sync.dma_start(iota_t[:], iota_d[:, :])
            meta_t = cpool.tile([P, n_cols], f32, tag="meta")
            nc.sync.dma_start(meta_t[:], meta_d[:, :])
            dis_t = cpool.tile([P, n_tiles], f32, tag="dis")
            nc.sync.dma_start(dis_t[:], dis_d[:, :])
            gidx_t = cpool.tile([P, e_pad // 16], mybir.dt.int16, tag="gidx")
            nc.sync.dma_start(gidx_t[:], gidx_d[:, :])

            gtiles = {}  # gid -> tile

            def issue_gather(gid):
                bb, s0, ln = gathers[gid]
                base = bb * bucket
                rows = min(bucket, n - base)
                gt = gpool.tile([P, (GATHER_CHUNK // P) * d], bf16, tag="gt",
                                name=f"gt{gid}")
                gt_3d = gt[:, :(ln // P) * d].rearrange("p (c d) -> p c d",
                                                        d=d)
                nc.gpsimd.dma_gather(
                    gt_3d,
                    x_d[base:base + rows, :],
                    gidx_t[:, s0 // 16:(s0 + ln) // 16],
                    ln, ln, d,
                    single_packet=True,
                    queue_num=gid % 4,
                )
                gtiles[gid] = gt

            next_gather = 0
            for blk in range(n_blocks):
                tiles_blk = range(blk * TILE_BLOCK,
                                  min((blk + 1) * TILE_BLOCK, n_tiles))
                # issue all gathers needed by this block
                last_col = max(c for jj in tiles_blk for (c, _b) in
                               tile_cols[jj])
                while next_gather < len(gathers):
                    bb, s0, ln = gathers[next_gather]
                    if s0 // P > last_col:
                        break
                    issue_gather(next_gather)
                    next_gather += 1

                for jj in tiles_blk:
                    cols = tile_cols[jj]
                    rows = min(P, npc - jj * P)
                    yt = ypool.tile([P, d], f32, tag="yt", name=f"yt{jj}")
                    if stage == "gather":
                        gid, col0 = gather_of_col[cols[0][0]]
                        nc.vector.tensor_copy(yt[:],
                                              gtiles[gid][:, :d])
                        nc.sync.dma_start(y_d[jj * P:jj * P + rows, :],
                                          yt[:rows, :])
                        continue
                    ohs = []
                    for si, (col, bb) in enumerate(cols):
                        oh = opool.tile([P, P], bf16, tag="oh",
                                        name=f"oh{col}")
                        nc.vector.tensor_scalar(
                            oh[:], iota_t[:],
                            meta_t[:, col:col + 1],
                            None,
                            mybir.AluOpType.is_equal,
                        )
                        ohs.append(oh)
                    if stage == "onehot":
                        nc.vector.tensor_copy(yt[:], ohs[0][:])
                        nc.sync.dma_start(y_d[jj * P:jj * P + rows, :],
                                          yt[:rows, :])
                        continue
                    pt = ppool.tile([P, d], f32, tag="ps", name=f"ps{jj}")
                    for si, (col, bb) in enumerate(cols):
                        gid, col0 = gather_of_col[col]
                        gt = gtiles[gid]
                        col_l = col - col0
                        nc.tensor.matmul(
                            pt[:],
                            lhsT=ohs[si][:, :],
                            rhs=gt[:, col_l * d:(col_l + 1) * d],
                            start=(si == 0),
                            stop=(si == len(cols) - 1),
                        )
                    if stage == "matmul":
                        nc.vector.tensor_copy(yt[:], pt[:])
                    else:
                        nc.scalar.activation(yt[:], pt[:], relu,
                                             scale=dis_t[:, jj:jj + 1])
                    nc.sync.dma_start(y_d[jj * P:jj * P + rows, :],
                                      yt[:rows, :])

    nc.compile()
    return nc


def _run(x, h, t, trace=False, stage="full"):
    import time
    t0 = time.monotonic()
    sched, per_core, x2b, iota = _preprocess(np.asarray(x), np.asarray(h),
                                             np.asarray(t))
    t1 = time.monotonic()
    print(f"[kernel] preprocess {t1 - t0:.1f}s  e_pad={sched['e_pad']} "
          f"cols={sched['n_cols']} gathers={len(sched['gathers'])}",
          flush=True)
    nc = _build_program(sched, stage=stage)
    t2 = time.monotonic()
    print(f"[kernel] build {t2 - t1:.1f}s", flush=True)
    in_maps = [
        {"x2": x2b, "iota": iota, "gidx": pc["gidx"], "meta": pc["meta"],
         "dis": pc["dis"]}
        for pc in per_core
    ]
    res = run_bass_kernel_spmd(nc, in_maps, core_ids=list(range(N_CORES)),
                               trace=trace)
    t3 = time.monotonic()
    print(f"[kernel] compile+run {t3 - t2:.1f}s", flush=True)
    y = np.concatenate([res.results[c]["y"] for c in range(N_CORES)], axis=0)
    return y, res


def kernel(x, h, t):
    y, _ = _run(np.asarray(x), np.asarray(h), np.asarray(t))
    return y


# revision 10
# speedup vs baseline: 3.1608x; 2.1881x over previous
"""PlainGCN message passing on 8 TRN2 NeuronCores.

Computation (reference):
    deg = bincount(h); dis = deg**-0.5; norm = dis[t]*dis[h]
    out = relu(segment_sum(norm[:,None] * x[h], t, N))

Strategy (v2):
  - Fold dis[h] into x host-side: x2 = dis[:,None]*x (bf16). Then
    out[t] = relu(dis[t] * segment_sum(x2[h], t)) — the per-edge norm
    disappears; dis[t] is applied once per dest tile, fused with the
    ReLU on ScalarE (per-partition scale).
  - Shard edges by destination: core c owns dest nodes
    [c*N/8, (c+1)*N/8); x2 replicated in HBM.
  - Per (dest tile j, source bucket b) the edges form a run padded to
    a multiple of 128 slots (shared SPMD schedule = max over cores).
    Pad slots carry gather idx -1 (skipped by the DMA when trailing)
    and tloc -1 (one-hot row = 0).
  - dma_gather x2 rows (256 B bf16) in <=1024-idx chunks,
    single_packet=True, round-robin over 4 SWDGE queues. The gather is
    Q7 descriptor-generation bound (~4 ns/idx), so everything else is
    scheduled to hide under it.
  - Segment-sum per dest tile: one-hot(tloc)=is_equal(iota) on DVE
    (bf16), TensorE matmul accumulation of full 128-slot columns into
    one PSUM bank per tile, then Relu(dis_t * psum) on ScalarE, DMA out.
"""

import numpy as np

import concourse.bacc as bacc
import concourse.mybir as mybir
import concourse.tile as tile
from concourse.bass_utils import run_bass_kernel_spmd
from concourse.library_config import mlp as mlp_lib

P = 128
N_NODES = 100000
D_FEAT = 128
N_CORES = 8
BUCKET_W = 25000     # source-bucket width (< 32768 so idx fits int16)
TILE_BLOCK = 4       # dest tiles per gather block
GATHER_CHUNK = 1024  # max idxs per dma_gather (single_packet safe limit)


def _preprocess(x, h, t):
    n, d = x.shape
    assert (n, d) == (N_NODES, D_FEAT)
    npc = n // N_CORES
    n_tiles = -(-npc // P)
    bucket = BUCKET_W
    n_buckets = -(-n // bucket)

    h = h.astype(np.int64)
    t = t.astype(np.int64)

    deg = np.bincount(h, minlength=n).astype(np.float64)
    dis = np.where(deg > 0, deg, 1.0) ** -0.5
    x2 = (x.astype(np.float64) * dis[:, None]).astype(np.float32)

    core = t // npc
    tloc = t - core * npc
    j = tloc // P
    tin = (tloc % P).astype(np.float32)
    b = h // BUCKET_W
    gidx_all = (h - b * BUCKET_W).astype(np.int16)

    counts = np.zeros((N_CORES, n_tiles, n_buckets), dtype=np.int64)
    np.add.at(counts, (core, j, b), 1)
    run_len = counts.max(axis=0)
    run_len = -(-run_len // P) * P  # full 128-slot columns only

    # stream order: blocks of TILE_BLOCK dest tiles; per block, per bucket,
    # the tiles' runs back to back.
    n_blocks = -(-n_tiles // TILE_BLOCK)
    run_start = np.zeros((n_tiles, n_buckets), dtype=np.int64)
    gathers = []  # (bucket, start, length) — <=GATHER_CHUNK, 128-aligned
    pos = 0
    for blk in range(n_blocks):
        tiles_blk = range(blk * TILE_BLOCK, min((blk + 1) * TILE_BLOCK, n_tiles))
        for bb in range(n_buckets):
            s0 = pos
            for jj in tiles_blk:
                run_start[jj, bb] = pos
                pos += int(run_len[jj, bb])
            # split [s0, pos) into gather chunks at run boundaries
            c0 = s0
            for jj in tiles_blk:
                r = int(run_len[jj, bb])
                end = run_start[jj, bb] + r
                if end - c0 > GATHER_CHUNK:
                    if run_start[jj, bb] > c0:
                        gathers.append((bb, c0, int(run_start[jj, bb] - c0)))
                    c0 = int(run_start[jj, bb])
                    while end - c0 > GATHER_CHUNK:
                        gathers.append((bb, c0, GATHER_CHUNK))
                        c0 += GATHER_CHUNK
            if pos > c0:
                gathers.append((bb, c0, int(pos - c0)))
    e_pad = pos
    n_cols = e_pad // P
    assert all(ln <= GATHER_CHUNK and ln % P == 0 for (_b, _s, ln) in gathers)

    # per-tile column lists: (col, bucket)
    tile_cols = []
    for jj in range(n_tiles):
        cols = []
        for bb in range(n_buckets):
            s, r = int(run_start[jj, bb]), int(run_len[jj, bb])
            cols.extend((c, bb) for c in range(s // P, (s + r) // P))
        tile_cols.append(cols)

    # per-core streams
    order_key = (j // TILE_BLOCK) * (n_buckets * n_tiles) + b * n_tiles + j
    per_core = []
    for c in range(N_CORES):
        sel = np.nonzero(core == c)[0]
        sel = sel[np.argsort(order_key[sel], kind="stable")]
        jj = j[sel]
        bb2 = b[sel]
        key = jj * n_buckets + bb2
        change = np.r_[True, key[1:] != key[:-1]]
        grp_id = np.cumsum(change) - 1
        first_pos = np.nonzero(change)[0]
        within = np.arange(len(sel)) - first_pos[grp_id]
        posn = run_start[jj, bb2] + within

        gi = np.full(e_pad, -1, dtype=np.int16)
        tf = np.full(e_pad, -1.0, dtype=np.float32)
        gi[posn] = gidx_all[sel]
        tf[posn] = tin[sel]
        # pad slots gather row 0 (harmless: their one-hot row is zero via
        # tloc=-1). Runtime -1-stripping desyncs the SWDGE ring bookkeeping
        # (device abort) — do not use negative indices.
        gi[gi == -1] = 0

        # wrap gather indices: [16, e/16] tiled x8 -> [128, e/16]
        wrap = np.tile(gi.reshape(e_pad // 16, 16).T, (8, 1)).astype(np.int16)

        meta = tf.reshape(n_cols, P).T.copy()  # [128, C] fp32

        # dis of this core's dest nodes, tiled [128, n_tiles]
        dnode = np.zeros(n_tiles * P, dtype=np.float32)
        dnode[:npc] = dis[c * npc:(c + 1) * npc].astype(np.float32)
        dis_t = dnode.reshape(n_tiles, P).T.copy()  # [128, n_tiles]

        per_core.append({"gidx": wrap, "meta": meta, "dis": dis_t})

    import ml_dtypes
    # iota strided at 2: value f at column 2f. The one-hot build reads it
    # with a step-2 AP, which forces the DVE into 1x single-port mode so it
    # never arbitrates for the POOL-shared SBUF port against the Q7s
    # (2-port DVE instructions stall for the whole gather otherwise).
    iota = np.zeros((P, 2 * P), dtype=np.float32)
    iota[:, ::2] = np.arange(P, dtype=np.float32)[None, :]
    x2b = x2.astype(ml_dtypes.bfloat16)

    sched = {
        "n": n, "d": d, "npc": npc, "n_tiles": n_tiles, "n_cols": n_cols,
        "e_pad": e_pad, "bucket": bucket, "n_buckets": n_buckets,
        "n_blocks": n_blocks, "gathers": gathers, "tile_cols": tile_cols,
        "run_start": run_start, "run_len": run_len,
    }
    return sched, per_core, x2b, iota


def _build_program(sched, stage="full"):
    n, d, npc = sched["n"], sched["d"], sched["npc"]
    n_tiles, n_cols, e_pad = sched["n_tiles"], sched["n_cols"], sched["e_pad"]
    bucket, n_buckets = sched["bucket"], sched["n_buckets"]
    n_blocks, gathers = sched["n_blocks"], sched["gathers"]
    tile_cols = sched["tile_cols"]

    nc = bacc.Bacc("TRN2", target_bir_lowering=False, debug=False,
                   num_devices=N_CORES, num_swdge_queues=4)
    f32 = mybir.dt.float32
    bf16 = mybir.dt.bfloat16
    x_d = nc.dram_tensor("x2", [n, d], bf16, kind="ExternalInput")
    iota_d = nc.dram_tensor("iota", [P, 2 * P], f32, kind="ExternalInput")
    gidx_d = nc.dram_tensor("gidx", [P, e_pad // 16], mybir.dt.int16,
                            kind="ExternalInput")
    meta_d = nc.dram_tensor("meta", [P, n_cols], f32, kind="ExternalInput")
    dis_d = nc.dram_tensor("dis", [P, n_tiles], f32, kind="ExternalInput")
    y_d = nc.dram_tensor("y", [npc, d], f32, kind="ExternalOutput")

    nc.gpsimd.load_library(mlp_lib)

    # gathers grouped by block for scheduling
    gather_of_col = {}
    for gid, (bb, s0, ln) in enumerate(gathers):
        for cc in range(s0 // P, (s0 + ln) // P):
            gather_of_col[cc] = (gid, s0 // P)

    relu = mybir.ActivationFunctionType.Relu

    with tile.TileContext(nc) as tc:
        with (
            tc.tile_pool(name="const", bufs=1) as cpool,
            tc.tile_pool(name="gather", bufs=10) as gpool,
            tc.tile_pool(name="onehot", bufs=12) as opool,
            tc.tile_pool(name="psum", bufs=8, space="PSUM") as ppool,
            tc.tile_pool(name="outs", bufs=4) as ypool,
        ):
            iota_t = cpool.tile([P, 2 * P], f32, tag="iota")
            nc.sync.dma_start(iota_t[:], iota_d[:, :])
            iota_s = iota_t.rearrange("p (f two) -> p f two", two=2)[:, :, 0]
            meta_t = cpool.tile([P, n_cols], f32, tag="meta")
            nc.sync.dma_start(meta_t[:], meta_d[:, :])
            dis_t = cpool.tile([P, n_tiles], f32, tag="dis")
            nc.sync.dma_start(dis_t[:], dis_d[:, :])
            gidx_t = cpool.tile([P, e_pad // 16], mybir.dt.int16, tag="gidx")
            nc.sync.dma_start(gidx_t[:], gidx_d[:, :])

            gtiles = {}  # gid -> tile

            def issue_gather(gid):
                bb, s0, ln = gathers[gid]
                base = bb * bucket
                rows = min(bucket, n - base)
                gt = gpool.tile([P, (GATHER_CHUNK // P) * d], bf16, tag="gt",
                                name=f"gt{gid}")
                gt_3d = gt[:, :(ln // P) * d].rearrange("p (c d) -> p c d",
                                                        d=d)
                nc.gpsimd.dma_gather(
                    gt_3d,
                    x_d[base:base + rows, :],
                    gidx_t[:, s0 // 16:(s0 + ln) // 16],
                    ln, ln, d,
                    single_packet=True,
                    queue_num=gid % 4,
                )
                gtiles[gid] = gt

            next_gather = 0
            for blk in range(n_blocks):
                tiles_blk = range(blk * TILE_BLOCK,
                                  min((blk + 1) * TILE_BLOCK, n_tiles))
                # issue all gathers needed by this block
                last_col = max(c for jj in tiles_blk for (c, _b) in
                               tile_cols[jj])
                while next_gather < len(gathers):
                    bb, s0, ln = gathers[next_gather]
                    if s0 // P > last_col:
                        break
                    issue_gather(next_gather)
                    next_gather += 1

                for jj in tiles_blk:
                    cols = tile_cols[jj]
                    rows = min(P, npc - jj * P)
                    yt = ypool.tile([P, d], f32, tag="yt", name=f"yt{jj}")
                    if stage == "gather":
                        gid, col0 = gather_of_col[cols[0][0]]
                        nc.vector.tensor_copy(yt[:],
                                              gtiles[gid][:, :d])
                        nc.sync.dma_start(y_d[jj * P:jj * P + rows, :],
                                          yt[:rows, :])
                        continue
                    ohs = []
                    for si, (col, bb) in enumerate(cols):
                        oh = opool.tile([P, P], bf16, tag="oh",
                                        name=f"oh{col}")
                        nc.vector.tensor_scalar(
                            oh[:], iota_s,
                            meta_t[:, col:col + 1],
                            None,
                            mybir.AluOpType.is_equal,
                        )
                        ohs.append(oh)
                    if stage == "onehot":
                        nc.vector.tensor_copy(yt[:], ohs[0][:])
                        nc.sync.dma_start(y_d[jj * P:jj * P + rows, :],
                                          yt[:rows, :])
                        continue
                    pt = ppool.tile([P, d], f32, tag="ps", name=f"ps{jj}")
                    for si, (col, bb) in enumerate(cols):
                        gid, col0 = gather_of_col[col]
                        gt = gtiles[gid]
                        col_l = col - col0
                        nc.tensor.matmul(
                            pt[:],
                            lhsT=ohs[si][:, :],
                            rhs=gt[:, col_l * d:(col_l + 1) * d],
                            start=(si == 0),
                            stop=(si == len(cols) - 1),
                        )
                    if stage == "matmul":
                        nc.vector.tensor_copy(yt[:], pt[:])
                    else:
                        nc.scalar.activation(yt[:], pt[:], relu,
                                             scale=dis_t[:, jj:jj + 1])
                    nc.sync.dma_start(y_d[jj * P:jj * P + rows, :],
                                      yt[:rows, :])

    nc.compile()
    return nc


def _run(x, h, t, trace=False, stage="full"):
    import time
    t0 = time.monotonic()
    sched, per_core, x2b, iota = _preprocess(np.asarray(x), np.asarray(h),
                                             np.asarray(t))
    t1 = time.monotonic()
    print(f"[kernel] preprocess {t1 - t0:.1f}s  e_pad={sched['e_pad']} "
          f"cols={sched['n_cols']} gathers={len(sched['gathers'])}",
          flush=True)
    nc = _build_program(sched, stage=stage)
    t2 = time.monotonic()
    print(f"[kernel] build {t2 - t1:.1f}s", flush=True)
    in_maps = [
        {"x2": x2b, "iota": iota, "gidx": pc["gidx"], "meta": pc["meta"],
         "dis": pc["dis"]}
        for pc in per_core
    ]
    res = run_bass_kernel_spmd(nc, in_maps, core_ids=list(range(N_CORES)),
                               trace=trace)
    t3 = time.monotonic()
    print(f"[kernel] compile+run {t3 - t2:.1f}s", flush=True)
    y = np.concatenate([res.results[c]["y"] for c in range(N_CORES)], axis=0)
    return y, res


def kernel(x, h, t):
    y, _ = _run(np.asarray(x), np.asarray(h), np.asarray(t))
    return y
